# revision 1
# baseline (speedup 1.0000x reference)
"""AdditiveAttention pooling kernel for 8 TRN2 NeuronCores.

reference:
    energy = tanh(lstm_output @ W_w.T + W_b)      # (B, S, H)
    scores = energy @ v_w                          # (B, S)
    scores = where(mask, scores, -1e9)
    weights = softmax(scores, axis=1)              # (B, S)
    context = einsum('bs,bsh->bh', weights, lstm_output)
    returns (context, weights)

Strategy: pure data-parallel over batch (B=64 -> 8 batches/core), no
collectives.  Single pass over x per core.  bf16 matmul inputs with fp32
PSUM accumulation.  |scores| <= ||v||_1 ~ 11.3 so softmax needs no
max-subtraction: w = exp(s + madd), Z = sum(w), out = w/Z.
"""

import sys

sys.path.insert(0, "/opt/trn_rl_repo")

import numpy as np
import ml_dtypes

import concourse.bass as bass
import concourse.tile as tile
from concourse import bacc, mybir
from concourse.bass_utils import run_bass_kernel_spmd

B, S, H = 64, 2048, 512
NCORES = 8
BPC = B // NCORES          # batches per core
TT = 512                   # tokens per tile
NT = S // TT               # tiles per batch
NC = H // 128              # 128-sized chunks of H

bf16 = ml_dtypes.bfloat16
DT_BF = mybir.dt.bfloat16
DT_F32 = mybir.dt.float32

_CACHE = {}


def build(bpc=BPC):
    nc = bacc.Bacc(None, target_bir_lowering=False)

    xt_d = nc.declare_dram_parameter("xt", [bpc, H, S], DT_BF, isOutput=False)
    xn_d = nc.declare_dram_parameter("xn", [bpc, S, H], DT_BF, isOutput=False)
    wt_d = nc.declare_dram_parameter("wt", [NC, 128, H], DT_BF, isOutput=False)
    bias_d = nc.declare_dram_parameter("bias", [NC, 128, 1], DT_F32, isOutput=False)
    vw_d = nc.declare_dram_parameter("vw", [NC, 128, 1], DT_BF, isOutput=False)
    madd_d = nc.declare_dram_parameter("madd", [bpc, S // 128, 128, 1], DT_F32, isOutput=False)
    ctx_d = nc.declare_dram_parameter("ctx", [bpc, H], DT_F32, isOutput=True)
    wts_d = nc.declare_dram_parameter("wts", [bpc, S], DT_F32, isOutput=True)

    TANH = mybir.ActivationFunctionType.Tanh
    EXP = mybir.ActivationFunctionType.Exp

    with tile.TileContext(nc) as tc:
        with (
            tc.tile_pool(name="const", bufs=1) as cpool,
            tc.tile_pool(name="xt", bufs=3) as xtp,
            tc.tile_pool(name="xn", bufs=3) as xnp,
            tc.tile_pool(name="madd", bufs=2) as mdp,
            tc.tile_pool(name="et", bufs=2) as etp,
            tc.tile_pool(name="wstage", bufs=2) as wsp,
            tc.tile_pool(name="small", bufs=2) as smp,
            tc.tile_pool(name="out", bufs=2) as outp,
            tc.tile_pool(name="psE", bufs=4, space="PSUM") as psEp,
            tc.tile_pool(name="psS", bufs=2, space="PSUM") as psSp,
            tc.tile_pool(name="psC", bufs=1, space="PSUM") as psCp,
            tc.tile_pool(name="psZ", bufs=1, space="PSUM") as psZp,
        ):
            # persistent constants
            wt_s = cpool.tile([128, NC, H], DT_BF)      # [h_p, hc, o]
            bias_s = cpool.tile([128, NC], DT_F32)      # [o_p, oc]
            v_s = cpool.tile([128, NC], DT_BF)          # [o_p, oc]
            ones_s = cpool.tile([128, 128], DT_BF)
            for hc in range(NC):
                nc.sync.dma_start(wt_s[:, hc, :], wt_d[hc])
            for oc in range(NC):
                nc.sync.dma_start(bias_s[:, oc : oc + 1], bias_d[oc])
                nc.sync.dma_start(v_s[:, oc : oc + 1], vw_d[oc])
            nc.vector.memset(ones_s[:], 1.0)

            for b in range(bpc):
                w_stage = wsp.tile([128, 4 * NT], DT_BF)   # [t_p, chunk]
                psC = psCp.tile([1, H], DT_F32)
                for j in range(NT):
                    xt_t = xtp.tile([128, NC, TT], DT_BF)   # [h_p, hc, t]
                    xn_t = xnp.tile([128, 4, H], DT_BF)     # [t_p, tc, h]
                    md_t = mdp.tile([128, 4], DT_F32)       # [t_p, tc]
                    for hc in range(NC):
                        nc.sync.dma_start(
                            xt_t[:, hc, :],
                            xt_d[b, hc * 128 : (hc + 1) * 128, j * TT : (j + 1) * TT],
                        )
                    for t in range(4):
                        nc.sync.dma_start(
                            xn_t[:, t, :],
                            xn_d[b, j * TT + t * 128 : j * TT + (t + 1) * 128, :],
                        )
                        nc.sync.dma_start(md_t[:, t : t + 1], madd_d[b, j * 4 + t])

                    # energy: E.T[o, t] = sum_h Wt[h, o] * xT[h, t]  (per o-chunk)
                    et_t = etp.tile([128, NC, TT], DT_BF)   # [o_p, oc, t]
                    for oc in range(NC):
                        psE = psEp.tile([128, TT], DT_F32)
                        for hc in range(NC):
                            nc.tensor.matmul(
                                psE[:],
                                wt_s[:, hc, oc * 128 : (oc + 1) * 128],
                                xt_t[:, hc, :],
                                start=(hc == 0),
                                stop=(hc == NC - 1),
                            )
                        nc.scalar.activation(
                            et_t[:, oc, :], psE[:], TANH, bias=bias_s[:, oc : oc + 1]
                        )

                    # scores: s[t] = sum_o v[o] * E.T[o, t]   -> (128t, 1) per t-chunk
                    psS = psSp.tile([128, 4], DT_F32)
                    for t in range(4):
                        for oc in range(NC):
                            nc.tensor.matmul(
                                psS[:, t : t + 1],
                                et_t[:, oc, t * 128 : (t + 1) * 128],
                                v_s[:, oc : oc + 1],
                                start=(oc == 0),
                                stop=(oc == NC - 1),
                            )
                    # w = exp(s + madd)   (bf16)
                    for t in range(4):
                        nc.scalar.activation(
                            w_stage[:, j * 4 + t : j * 4 + t + 1],
                            psS[:, t : t + 1],
                            EXP,
                            bias=md_t[:, t : t + 1],
                        )
                    # context: ctx[h] += sum_t w[t] * xn[t, h]
                    for t in range(4):
                        nc.tensor.matmul(
                            psC[:],
                            w_stage[:, j * 4 + t : j * 4 + t + 1],
                            xn_t[:, t, :],
                            start=(j == 0 and t == 0),
                            stop=(j == NT - 1 and t == 3),
                        )

                # batch epilogue: Z, 1/Z, scale, store
                psZ = psZp.tile([128, 4 * NT], DT_F32)
                nc.tensor.matmul(psZ[:], ones_s[:], w_stage[:], start=True, stop=True)
                zrep = smp.tile([128, 1], DT_F32)
                nc.vector.tensor_reduce(
                    zrep[:], psZ[:], axis=mybir.AxisListType.X, op=mybir.AluOpType.add
                )
                rz = smp.tile([128, 1], DT_F32)
                nc.vector.reciprocal(rz[:], zrep[:])
                wout = outp.tile([128, 4 * NT], DT_F32)
                nc.vector.tensor_scalar_mul(wout[:], w_stage[:], rz[:])
                ctxout = outp.tile([1, H], DT_F32)
                nc.vector.tensor_scalar_mul(ctxout[:], psC[:], rz[0:1, :])
                nc.sync.dma_start(
                    wts_d[b].rearrange("(c p) -> p c", p=128), wout[:]
                )
                nc.sync.dma_start(ctx_d[b : b + 1, :], ctxout[:])

    nc.compile()
    return nc


def _prep_inputs(lstm_output, mask, W_w, W_b, v_w):
    x = np.asarray(lstm_output, dtype=np.float32)
    xn = x.astype(bf16)                                        # (B, S, H)
    xt = np.ascontiguousarray(x.transpose(0, 2, 1)).astype(bf16)  # (B, H, S)
    wt = np.ascontiguousarray(np.asarray(W_w, np.float32).T.reshape(NC, 128, H)).astype(bf16)
    biasc = np.ascontiguousarray(np.asarray(W_b, np.float32).reshape(NC, 128, 1))
    vwc = np.ascontiguousarray(np.asarray(v_w, np.float32).reshape(NC, 128, 1)).astype(bf16)
    madd = np.where(np.asarray(mask), np.float32(0.0), np.float32(-1e9)).astype(np.float32)
    madd = np.ascontiguousarray(madd.reshape(B, S // 128, 128, 1))

    in_maps = []
    for c in range(NCORES):
        sl = slice(c * BPC, (c + 1) * BPC)
        in_maps.append(
            {
                "xt": np.ascontiguousarray(xt[sl]),
                "xn": np.ascontiguousarray(xn[sl]),
                "wt": wt,
                "bias": biasc,
                "vw": vwc,
                "madd": np.ascontiguousarray(madd[sl]),
            }
        )
    return in_maps


def kernel(lstm_output, mask, W_w, W_b, v_w):
    if "nc" not in _CACHE:
        _CACHE["nc"] = build()
    nc = _CACHE["nc"]
    in_maps = _prep_inputs(lstm_output, mask, W_w, W_b, v_w)
    res = run_bass_kernel_spmd(nc, in_maps, core_ids=list(range(NCORES)))
    ctx = np.concatenate([res.results[i]["ctx"] for i in range(NCORES)], axis=0)
    wts = np.concatenate([res.results[i]["wts"] for i in range(NCORES)], axis=0)
    return ctx.astype(np.float32), wts.astype(np.float32)


# revision 4
# speedup vs baseline: 305.9715x; 305.9715x over previous
"""AdditiveAttention pooling kernel for 8 TRN2 NeuronCores.

reference:
    energy = tanh(lstm_output @ W_w.T + W_b)      # (B, S, H)
    scores = energy @ v_w                          # (B, S)
    scores = where(mask, scores, -1e9)
    weights = softmax(scores, axis=1)              # (B, S)
    context = einsum('bs,bsh->bh', weights, lstm_output)
    returns (context, weights)

Strategy: pure data-parallel over batch (B=64 -> 8 batches/core), no
collectives.  Single pass over x per core.  bf16 matmul inputs with fp32
PSUM accumulation.  |scores| <= ||v||_1 ~ 11.3 so softmax needs no
max-subtraction: w = exp(s + madd), Z = sum(w), out = w/Z.
"""

import sys

sys.path.insert(0, "/opt/trn_rl_repo")

import numpy as np
import ml_dtypes

import concourse.bass as bass
import concourse.tile as tile
from concourse import bacc, mybir
from concourse.bass_utils import run_bass_kernel_spmd

B, S, H = 64, 2048, 512
NCORES = 8
BPC = B // NCORES          # batches per core
TT = 512                   # tokens per tile
NT = S // TT               # tiles per batch
NC = H // 128              # 128-sized chunks of H

bf16 = ml_dtypes.bfloat16
DT_BF = mybir.dt.bfloat16
DT_F32 = mybir.dt.float32

_CACHE = {}


def build(bpc=BPC, repeat=1):
    nc = bacc.Bacc(None, target_bir_lowering=False)

    xt_d = nc.declare_dram_parameter("xt", [bpc, H, S], DT_BF, isOutput=False)
    xn_d = nc.declare_dram_parameter("xn", [bpc, S, H], DT_BF, isOutput=False)
    wt_d = nc.declare_dram_parameter("wt", [NC, 128, H], DT_BF, isOutput=False)
    bias_d = nc.declare_dram_parameter("bias", [NC, 128], DT_F32, isOutput=False)
    vw_d = nc.declare_dram_parameter("vw", [NC, 128], DT_BF, isOutput=False)
    madd_d = nc.declare_dram_parameter("madd", [bpc, S // 128, 128], DT_F32, isOutput=False)
    ctx_d = nc.declare_dram_parameter("ctx", [bpc, H], DT_F32, isOutput=True)
    wts_d = nc.declare_dram_parameter("wts", [bpc, S], DT_F32, isOutput=True)

    TANH = mybir.ActivationFunctionType.Tanh
    EXP = mybir.ActivationFunctionType.Exp

    with tile.TileContext(nc) as tc:
        with (
            tc.tile_pool(name="const", bufs=1) as cpool,
            tc.tile_pool(name="xt", bufs=3) as xtp,
            tc.tile_pool(name="xn", bufs=3) as xnp,
            tc.tile_pool(name="madd", bufs=2) as mdp,
            tc.tile_pool(name="et", bufs=2) as etp,
            tc.tile_pool(name="wstage", bufs=2) as wsp,
            tc.tile_pool(name="small", bufs=2) as smp,
            tc.tile_pool(name="out", bufs=2) as outp,
            tc.tile_pool(name="psE", bufs=4, space="PSUM") as psEp,
            tc.tile_pool(name="psS", bufs=2, space="PSUM") as psSp,
            tc.tile_pool(name="psC", bufs=1, space="PSUM") as psCp,
            tc.tile_pool(name="psZ", bufs=1, space="PSUM") as psZp,
        ):
            # persistent constants
            wt_s = cpool.tile([128, NC, H], DT_BF)      # [h_p, hc, o]
            bias_s = cpool.tile([128, NC], DT_F32)      # [o_p, oc]
            v_s = cpool.tile([128, NC], DT_BF)          # [o_p, oc]
            ones_s = cpool.tile([128, 128], DT_BF)
            nc.sync.dma_start(wt_s[:], wt_d[:].rearrange("c p o -> p c o"))
            nc.sync.dma_start(bias_s[:], bias_d[:].rearrange("c p -> p c"))
            nc.sync.dma_start(v_s[:], vw_d[:].rearrange("c p -> p c"))
            nc.vector.memset(ones_s[:], 1.0)

            for b in [bb for _ in range(repeat) for bb in range(bpc)]:
                w_stage = wsp.tile([128, 4 * NT], DT_BF)   # [t_p, chunk]
                psC = psCp.tile([1, H], DT_F32)
                for j in range(NT):
                    xt_t = xtp.tile([128, NC, TT], DT_BF)   # [h_p, hc, t]
                    xn_t = xnp.tile([128, 4, H], DT_BF)     # [t_p, tc, h]
                    md_t = mdp.tile([128, 4], DT_F32)       # [t_p, tc]
                    nc.sync.dma_start(
                        xt_t[:],
                        xt_d[b, :, j * TT : (j + 1) * TT].rearrange(
                            "(c p) t -> p c t", p=128
                        ),
                    )
                    nc.gpsimd.dma_start(
                        xn_t[:],
                        xn_d[b, j * TT : (j + 1) * TT, :].rearrange(
                            "(c p) h -> p c h", p=128
                        ),
                    )
                    nc.gpsimd.dma_start(
                        md_t[:], madd_d[b, j * 4 : (j + 1) * 4, :].rearrange("c p -> p c")
                    )

                    # energy: E.T[o, t] = sum_h Wt[h, o] * xT[h, t]  (per o-chunk)
                    et_t = etp.tile([128, NC, TT], DT_BF)   # [o_p, oc, t]
                    for oc in range(NC):
                        psE = psEp.tile([128, TT], DT_F32)
                        for hc in range(NC):
                            nc.tensor.matmul(
                                psE[:],
                                wt_s[:, hc, oc * 128 : (oc + 1) * 128],
                                xt_t[:, hc, :],
                                start=(hc == 0),
                                stop=(hc == NC - 1),
                            )
                        nc.scalar.activation(
                            et_t[:, oc, :], psE[:], TANH, bias=bias_s[:, oc : oc + 1]
                        )

                    # scores: s[t] = sum_o v[o] * E.T[o, t]   -> (128t, 1) per t-chunk
                    psS = psSp.tile([128, 4], DT_F32)
                    for t in range(4):
                        for oc in range(NC):
                            nc.tensor.matmul(
                                psS[:, t : t + 1],
                                et_t[:, oc, t * 128 : (t + 1) * 128],
                                v_s[:, oc : oc + 1],
                                start=(oc == 0),
                                stop=(oc == NC - 1),
                            )
                    # w = exp(s + madd)   (bf16)
                    for t in range(4):
                        nc.scalar.activation(
                            w_stage[:, j * 4 + t : j * 4 + t + 1],
                            psS[:, t : t + 1],
                            EXP,
                            bias=md_t[:, t : t + 1],
                        )
                    # context: ctx[h] += sum_t w[t] * xn[t, h]
                    for t in range(4):
                        nc.tensor.matmul(
                            psC[:],
                            w_stage[:, j * 4 + t : j * 4 + t + 1],
                            xn_t[:, t, :],
                            start=(j == 0 and t == 0),
                            stop=(j == NT - 1 and t == 3),
                        )

                # batch epilogue: Z, 1/Z, scale, store
                psZ = psZp.tile([128, 4 * NT], DT_F32)
                nc.tensor.matmul(psZ[:], ones_s[:], w_stage[:], start=True, stop=True)
                zrep = smp.tile([128, 1], DT_F32)
                nc.vector.tensor_reduce(
                    zrep[:], psZ[:], axis=mybir.AxisListType.X, op=mybir.AluOpType.add
                )
                rz = smp.tile([128, 1], DT_F32)
                nc.vector.reciprocal(rz[:], zrep[:])
                wout = outp.tile([128, 4 * NT], DT_F32)
                nc.vector.tensor_scalar_mul(wout[:], w_stage[:], rz[:])
                ctxout = outp.tile([1, H], DT_F32)
                nc.vector.tensor_scalar_mul(ctxout[:], psC[:], rz[0:1, :])
                nc.sync.dma_start(
                    wts_d[b].rearrange("(c p) -> p c", p=128), wout[:]
                )
                nc.sync.dma_start(ctx_d[b : b + 1, :], ctxout[:])

    nc.compile()
    return nc


def _prep_inputs(lstm_output, mask, W_w, W_b, v_w):
    x = np.asarray(lstm_output, dtype=np.float32)
    xn = x.astype(bf16)                                        # (B, S, H)
    xt = np.ascontiguousarray(x.transpose(0, 2, 1)).astype(bf16)  # (B, H, S)
    wt = np.ascontiguousarray(np.asarray(W_w, np.float32).T.reshape(NC, 128, H)).astype(bf16)
    biasc = np.ascontiguousarray(np.asarray(W_b, np.float32).reshape(NC, 128))
    vwc = np.ascontiguousarray(np.asarray(v_w, np.float32).reshape(NC, 128)).astype(bf16)
    madd = np.where(np.asarray(mask), np.float32(0.0), np.float32(-1e9)).astype(np.float32)
    madd = np.ascontiguousarray(madd.reshape(B, S // 128, 128))

    in_maps = []
    for c in range(NCORES):
        sl = slice(c * BPC, (c + 1) * BPC)
        in_maps.append(
            {
                "xt": np.ascontiguousarray(xt[sl]),
                "xn": np.ascontiguousarray(xn[sl]),
                "wt": wt,
                "bias": biasc,
                "vw": vwc,
                "madd": np.ascontiguousarray(madd[sl]),
            }
        )
    return in_maps


def kernel(lstm_output, mask, W_w, W_b, v_w):
    if "nc" not in _CACHE:
        _CACHE["nc"] = build()
    nc = _CACHE["nc"]
    in_maps = _prep_inputs(lstm_output, mask, W_w, W_b, v_w)
    res = run_bass_kernel_spmd(nc, in_maps, core_ids=list(range(NCORES)))
    ctx = np.concatenate([res.results[i]["ctx"] for i in range(NCORES)], axis=0)
    wts = np.concatenate([res.results[i]["wts"] for i in range(NCORES)], axis=0)
    return ctx.astype(np.float32), wts.astype(np.float32)


# revision 7
# speedup vs baseline: 310.6403x; 1.0153x over previous
"""AdditiveAttention pooling kernel for 8 TRN2 NeuronCores.

reference:
    energy = tanh(lstm_output @ W_w.T + W_b)      # (B, S, H)
    scores = energy @ v_w                          # (B, S)
    scores = where(mask, scores, -1e9)
    weights = softmax(scores, axis=1)              # (B, S)
    context = einsum('bs,bsh->bh', weights, lstm_output)
    returns (context, weights)

Strategy: pure data-parallel over batch (B=64 -> 8 batches/core), no
collectives.  Single pass over x per core.  bf16 matmul inputs with fp32
PSUM accumulation.  |scores| <= ||v||_1 ~ 11.3 so softmax needs no
max-subtraction: w = exp(s + madd), Z = sum(w), out = w/Z.

Per batch (S=2048), processed in 2 half-batches of HT=1024 tokens:
  energy   psE[o_chunk] (128, 1024) += Wt[hc,oc].T @ xT[hc]   (8 MMs/oc)
  tanh     et[:, oc, :] = tanh(psE + bias[oc])                (1 long ACT/oc)
  score    psS[:, tc] += et[:, oc, 128tc:].T @ v[oc]          (32 MMs)
  mask     sc = psS + madd   (DVE)
  exp      w_stage[:, 8 cols] = exp(sc)                       (1 ACT)
  context  psC (1, 512) += w_col.T @ xn[tc]                   (8 MMs)
Epilogue per batch: Z = colsum(w_stage) via ones-matmul, 1/Z, scale, store.
"""

import sys

sys.path.insert(0, "/opt/trn_rl_repo")

import numpy as np
import ml_dtypes

import concourse.bass as bass
import concourse.tile as tile
from concourse import bacc, mybir
from concourse.bass_utils import run_bass_kernel_spmd

B, S, H = 64, 2048, 512
NCORES = 8
BPC = B // NCORES          # batches per core
HT = 1024                  # tokens per half-batch
NH = S // HT               # half-batches per batch (2)
NC = H // 128              # 128-sized chunks of H

bf16 = ml_dtypes.bfloat16
DT_BF = mybir.dt.bfloat16
DT_F32 = mybir.dt.float32

_CACHE = {}


def build(bpc=BPC, repeat=1):
    nc = bacc.Bacc(None, target_bir_lowering=False)

    # host-prearranged layouts (p = SBUF partition):
    #   xtp[b, p, hc, s] = x[b, s, hc*128+p]     (moving operand of energy MM)
    #   xnp[b, p, c, h]  = x[b, c*128+p, h]      (moving operand of ctx MM)
    xtp_d = nc.declare_dram_parameter("xtp", [bpc, 128, NC, S], DT_BF, isOutput=False)
    xnp_d = nc.declare_dram_parameter("xnp", [bpc, 128, S // 128, H], DT_BF, isOutput=False)
    wt_d = nc.declare_dram_parameter("wt", [NC, 128, H], DT_BF, isOutput=False)
    bias_d = nc.declare_dram_parameter("bias", [NC, 128], DT_F32, isOutput=False)
    vw_d = nc.declare_dram_parameter("vw", [NC, 128], DT_BF, isOutput=False)
    madd_d = nc.declare_dram_parameter("madd", [bpc, 128, S // 128], DT_F32, isOutput=False)
    ctx_d = nc.declare_dram_parameter("ctx", [bpc, H], DT_F32, isOutput=True)
    wts_d = nc.declare_dram_parameter("wts", [bpc, S], DT_F32, isOutput=True)

    TANH = mybir.ActivationFunctionType.Tanh
    EXP = mybir.ActivationFunctionType.Exp
    NCH = HT // 128  # token chunks per half-batch (8)

    with tile.TileContext(nc) as tc:
        with (
            tc.tile_pool(name="const", bufs=1) as cpool,
            tc.tile_pool(name="xt", bufs=3) as xtp,
            tc.tile_pool(name="xn", bufs=3) as xnp,
            tc.tile_pool(name="madd", bufs=2) as mdp,
            tc.tile_pool(name="et", bufs=2) as etp,
            tc.tile_pool(name="wstage", bufs=2) as wsp,
            tc.tile_pool(name="small", bufs=3) as smp,
            tc.tile_pool(name="out", bufs=2) as outp,
            tc.tile_pool(name="psE", bufs=2, space="PSUM") as psEp,   # 2 banks each
            tc.tile_pool(name="psS", bufs=2, space="PSUM") as psSp,   # 1 bank each
            tc.tile_pool(name="psC", bufs=2, space="PSUM") as psCp,   # 1 bank each
        ):
            # persistent constants
            wt_s = cpool.tile([128, NC, H], DT_BF)      # [h_p, hc, o]
            bias_s = cpool.tile([128, NC], DT_F32)      # [o_p, oc]
            v_s = cpool.tile([128, NC], DT_BF)          # [o_p, oc]
            ones_s = cpool.tile([128, 128], DT_BF)
            ones1_s = cpool.tile([1, 128], DT_F32)
            nc.sync.dma_start(wt_s[:], wt_d[:].rearrange("c p o -> p c o"))
            nc.sync.dma_start(bias_s[:], bias_d[:].rearrange("c p -> p c"))
            nc.sync.dma_start(v_s[:], vw_d[:].rearrange("c p -> p c"))
            nc.vector.memset(ones_s[:], 1.0)
            nc.vector.memset(ones1_s[:], 1.0)

            for b in [bb for _ in range(repeat) for bb in range(bpc)]:
                w_stage = wsp.tile([128, S // 128], DT_BF)   # [t_p, chunk]
                psC = psCp.tile([1, H], DT_F32)
                md_t = mdp.tile([128, S // 128], DT_F32)     # [t_p, chunk]
                nc.sync.dma_start(md_t[:], madd_d[b])
                for half in range(NH):
                    xt_h = xtp.tile([128, NC, HT], DT_BF)    # [h_p, hc, t]
                    xn_h = xnp.tile([128, NCH, H], DT_BF)    # [t_p, tc, h]
                    nc.sync.dma_start(
                        xt_h[:], xtp_d[b, :, :, half * HT : (half + 1) * HT]
                    )
                    nc.gpsimd.dma_start(
                        xn_h[:], xnp_d[b, :, half * NCH : (half + 1) * NCH, :]
                    )

                    # energy + tanh (one long ACT per o-chunk)
                    et_h = etp.tile([128, NC, HT], DT_BF)    # [o_p, oc, t]
                    for oc in range(NC):
                        psE = psEp.tile([128, HT], DT_F32)
                        for hc in range(NC):
                            for jh in range(HT // 512):
                                nc.tensor.matmul(
                                    psE[:, jh * 512 : (jh + 1) * 512],
                                    wt_s[:, hc, oc * 128 : (oc + 1) * 128],
                                    xt_h[:, hc, jh * 512 : (jh + 1) * 512],
                                    start=(hc == 0),
                                    stop=(hc == NC - 1),
                                )
                        nc.scalar.activation(
                            et_h[:, oc, :], psE[:], TANH, bias=bias_s[:, oc : oc + 1]
                        )

                    # scores for 8 token-chunks
                    psS = psSp.tile([128, NCH], DT_F32)
                    for t in range(NCH):
                        for oc in range(NC):
                            nc.tensor.matmul(
                                psS[:, t : t + 1],
                                et_h[:, oc, t * 128 : (t + 1) * 128],
                                v_s[:, oc : oc + 1],
                                start=(oc == 0),
                                stop=(oc == NC - 1),
                            )
                    # mask-add on DVE, then one batched exp
                    sc_m = smp.tile([128, NCH], DT_F32)
                    nc.vector.tensor_add(
                        sc_m[:], psS[:], md_t[:, half * NCH : (half + 1) * NCH]
                    )
                    nc.scalar.activation(
                        w_stage[:, half * NCH : (half + 1) * NCH], sc_m[:], EXP
                    )
                    # context accumulation
                    for t in range(NCH):
                        c = half * NCH + t
                        nc.tensor.matmul(
                            psC[:],
                            w_stage[:, c : c + 1],
                            xn_h[:, t, :],
                            start=(c == 0),
                            stop=(c == S // 128 - 1),
                        )

                # batch epilogue: Z, 1/Z, scale, store
                psZ = psSp.tile([1, S // 128], DT_F32, tag="psS")
                nc.tensor.matmul(psZ[:], ones_s[:, 0:1], w_stage[:], start=True, stop=True)
                z1 = smp.tile([1, 1], DT_F32)
                nc.vector.tensor_reduce(
                    z1[:], psZ[:], axis=mybir.AxisListType.X, op=mybir.AluOpType.add
                )
                psZb = psSp.tile([128, 1], DT_F32, tag="psS")
                nc.tensor.matmul(psZb[:], ones1_s[:], z1[:], start=True, stop=True)
                rz = smp.tile([128, 1], DT_F32)
                nc.vector.reciprocal(rz[:], psZb[:])
                wout = outp.tile([128, S // 128], DT_F32)
                nc.vector.tensor_scalar_mul(wout[:], w_stage[:], rz[:])
                ctxout = outp.tile([1, H], DT_F32)
                nc.vector.tensor_scalar_mul(ctxout[:], psC[:], rz[0:1, :])
                nc.sync.dma_start(
                    wts_d[b].rearrange("(c p) -> p c", p=128), wout[:]
                )
                nc.sync.dma_start(ctx_d[b : b + 1, :], ctxout[:])

    nc.compile()
    return nc


def _prep_inputs(lstm_output, mask, W_w, W_b, v_w):
    x = np.asarray(lstm_output, dtype=np.float32)
    xb = x.astype(bf16)                                   # (B, S, H)
    # xtp[b, p, hc, s] = x[b, s, hc*128+p]
    xtp = np.ascontiguousarray(
        xb.reshape(B, S, NC, 128).transpose(0, 3, 2, 1)
    )
    # xnp[b, p, c, h] = x[b, c*128+p, h]
    xnp = np.ascontiguousarray(
        xb.reshape(B, S // 128, 128, H).transpose(0, 2, 1, 3)
    )
    wt = np.ascontiguousarray(np.asarray(W_w, np.float32).T.reshape(NC, 128, H)).astype(bf16)
    biasc = np.ascontiguousarray(np.asarray(W_b, np.float32).reshape(NC, 128))
    vwc = np.ascontiguousarray(np.asarray(v_w, np.float32).reshape(NC, 128)).astype(bf16)
    madd = np.where(np.asarray(mask), np.float32(0.0), np.float32(-1e9)).astype(np.float32)
    # madd_d[b, p, c] = madd[b, c*128+p]
    madd = np.ascontiguousarray(madd.reshape(B, S // 128, 128).transpose(0, 2, 1))

    in_maps = []
    for c in range(NCORES):
        sl = slice(c * BPC, (c + 1) * BPC)
        in_maps.append(
            {
                "xtp": np.ascontiguousarray(xtp[sl]),
                "xnp": np.ascontiguousarray(xnp[sl]),
                "wt": wt,
                "bias": biasc,
                "vw": vwc,
                "madd": np.ascontiguousarray(madd[sl]),
            }
        )
    return in_maps


def kernel(lstm_output, mask, W_w, W_b, v_w):
    if "nc" not in _CACHE:
        _CACHE["nc"] = build()
    nc = _CACHE["nc"]
    in_maps = _prep_inputs(lstm_output, mask, W_w, W_b, v_w)
    res = run_bass_kernel_spmd(nc, in_maps, core_ids=list(range(NCORES)))
    ctx = np.concatenate([res.results[i]["ctx"] for i in range(NCORES)], axis=0)
    wts = np.concatenate([res.results[i]["wts"] for i in range(NCORES)], axis=0)
    return ctx.astype(np.float32), wts.astype(np.float32)


# revision 8
# speedup vs baseline: 381.2018x; 1.2271x over previous
"""AdditiveAttention pooling kernel for 8 TRN2 NeuronCores.

reference:
    energy = tanh(lstm_output @ W_w.T + W_b)      # (B, S, H)
    scores = energy @ v_w                          # (B, S)
    scores = where(mask, scores, -1e9)
    weights = softmax(scores, axis=1)              # (B, S)
    context = einsum('bs,bsh->bh', weights, lstm_output)
    returns (context, weights)

Strategy: pure data-parallel over batch (B=64 -> 8 batches/core), no
collectives.  Single pass over x per core.  bf16 matmul inputs with fp32
PSUM accumulation.  |scores| <= ||v||_1 ~ 11.3 so softmax needs no
max-subtraction: w = exp(s + madd), Z = sum(w), out = w/Z.

Per batch (S=2048), processed in 2 half-batches of HT=1024 tokens:
  energy   psE[o_chunk] (128, 1024) += Wt[hc,oc].T @ xT[hc]   (8 MMs/oc)
  tanh     et[:, oc, :] = tanh(psE + bias[oc])                (1 long ACT/oc)
  score    psS[:, tc] += et[:, oc, 128tc:].T @ v[oc]          (32 MMs)
  mask     sc = psS + madd   (DVE)
  exp      w_stage[:, 8 cols] = exp(sc)                       (1 ACT)
  context  psC (1, 512) += w_col.T @ xn[tc]                   (8 MMs)
Epilogue per batch: Z = colsum(w_stage) via ones-matmul, 1/Z, scale, store.
"""

import sys

sys.path.insert(0, "/opt/trn_rl_repo")

import numpy as np
import ml_dtypes

import concourse.bass as bass
import concourse.tile as tile
from concourse import bacc, mybir
from concourse.bass_utils import run_bass_kernel_spmd

B, S, H = 64, 2048, 512
NCORES = 8
BPC = B // NCORES          # batches per core
HT = 1024                  # tokens per half-batch
NH = S // HT               # half-batches per batch (2)
NC = H // 128              # 128-sized chunks of H

bf16 = ml_dtypes.bfloat16
DT_BF = mybir.dt.bfloat16
DT_F32 = mybir.dt.float32

_CACHE = {}


def build(bpc=BPC, repeat=1):
    nc = bacc.Bacc(None, target_bir_lowering=False)

    # host-prearranged layouts (p = SBUF partition):
    #   xtp[b, p, hc, s] = x[b, s, hc*128+p]     (moving operand of energy MM)
    #   xnp[b, p, c, h]  = x[b, c*128+p, h]      (moving operand of ctx MM)
    xtp_d = nc.declare_dram_parameter("xtp", [bpc, 128, NC, S], DT_BF, isOutput=False)
    xnp_d = nc.declare_dram_parameter("xnp", [bpc, 128, S // 128, H], DT_BF, isOutput=False)
    wt_d = nc.declare_dram_parameter("wt", [NC, 128, H], DT_BF, isOutput=False)
    bias_d = nc.declare_dram_parameter("bias", [NC, 128], DT_F32, isOutput=False)
    vw_d = nc.declare_dram_parameter("vw", [NC, 128], DT_BF, isOutput=False)
    madd_d = nc.declare_dram_parameter("madd", [bpc, 128, S // 128], DT_F32, isOutput=False)
    ctx_d = nc.declare_dram_parameter("ctx", [bpc, H], DT_F32, isOutput=True)
    wts_d = nc.declare_dram_parameter("wts", [bpc, S], DT_F32, isOutput=True)

    TANH = mybir.ActivationFunctionType.Tanh
    EXP = mybir.ActivationFunctionType.Exp
    NCH = HT // 128  # token chunks per half-batch (8)

    with tile.TileContext(nc) as tc:
        with (
            tc.tile_pool(name="const", bufs=1) as cpool,
            tc.tile_pool(name="xt", bufs=3) as xtp,
            tc.tile_pool(name="xn", bufs=3) as xnp,
            tc.tile_pool(name="madd", bufs=2) as mdp,
            tc.tile_pool(name="et", bufs=2) as etp,
            tc.tile_pool(name="wstage", bufs=2) as wsp,
            tc.tile_pool(name="small", bufs=3) as smp,
            tc.tile_pool(name="out", bufs=2) as outp,
            tc.tile_pool(name="psE", bufs=2, space="PSUM") as psEp,   # 2 banks each
            tc.tile_pool(name="psS", bufs=2, space="PSUM") as psSp,   # 1 bank each
            tc.tile_pool(name="psC", bufs=2, space="PSUM") as psCp,   # 1 bank each
        ):
            # persistent constants
            wt_s = cpool.tile([128, NC, H], DT_BF)      # [h_p, hc, o]
            bias_s = cpool.tile([128, NC], DT_F32)      # [o_p, oc]
            v_s = cpool.tile([128, NC], DT_BF)          # [o_p, oc]
            ones_s = cpool.tile([128, 128], DT_BF)
            ones1_s = cpool.tile([1, 128], DT_F32)
            nc.sync.dma_start(wt_s[:], wt_d[:].rearrange("c p o -> p c o"))
            nc.sync.dma_start(bias_s[:], bias_d[:].rearrange("c p -> p c"))
            nc.sync.dma_start(v_s[:], vw_d[:].rearrange("c p -> p c"))
            nc.vector.memset(ones_s[:], 1.0)
            nc.vector.memset(ones1_s[:], 1.0)

            for b in [bb for _ in range(repeat) for bb in range(bpc)]:
                w_stage = wsp.tile([128, S // 128], DT_BF)   # [t_p, chunk]
                psC = psCp.tile([1, H], DT_F32)
                md_t = mdp.tile([128, S // 128], DT_F32)     # [t_p, chunk]
                nc.sync.dma_start(md_t[:], madd_d[b])
                # full-batch loads: 16KB contiguous per partition on both sides
                xt_b = xtp.tile([128, NC, S], DT_BF)         # [h_p, hc, s]
                xn_b = xnp.tile([128, S // 128, H], DT_BF)   # [t_p, c, h]
                nc.sync.dma_start(xt_b[:], xtp_d[b])
                nc.gpsimd.dma_start(xn_b[:], xnp_d[b])
                for half in range(NH):
                    t0 = half * HT

                    # energy + tanh (one long ACT per o-chunk)
                    et_h = etp.tile([128, NC, HT], DT_BF)    # [o_p, oc, t]
                    for oc in range(NC):
                        psE = psEp.tile([128, HT], DT_F32)
                        for hc in range(NC):
                            for jh in range(HT // 512):
                                nc.tensor.matmul(
                                    psE[:, jh * 512 : (jh + 1) * 512],
                                    wt_s[:, hc, oc * 128 : (oc + 1) * 128],
                                    xt_b[:, hc, t0 + jh * 512 : t0 + (jh + 1) * 512],
                                    start=(hc == 0),
                                    stop=(hc == NC - 1),
                                )
                        nc.scalar.activation(
                            et_h[:, oc, :], psE[:], TANH, bias=bias_s[:, oc : oc + 1]
                        )

                    # scores for 8 token-chunks
                    psS = psSp.tile([128, NCH], DT_F32)
                    for t in range(NCH):
                        for oc in range(NC):
                            nc.tensor.matmul(
                                psS[:, t : t + 1],
                                et_h[:, oc, t * 128 : (t + 1) * 128],
                                v_s[:, oc : oc + 1],
                                start=(oc == 0),
                                stop=(oc == NC - 1),
                            )
                    # mask-add on DVE, then one batched exp
                    sc_m = smp.tile([128, NCH], DT_F32)
                    nc.vector.tensor_add(
                        sc_m[:], psS[:], md_t[:, half * NCH : (half + 1) * NCH]
                    )
                    nc.scalar.activation(
                        w_stage[:, half * NCH : (half + 1) * NCH], sc_m[:], EXP
                    )
                    # context accumulation
                    for t in range(NCH):
                        c = half * NCH + t
                        nc.tensor.matmul(
                            psC[:],
                            w_stage[:, c : c + 1],
                            xn_b[:, c, :],
                            start=(c == 0),
                            stop=(c == S // 128 - 1),
                        )

                # batch epilogue: Z, 1/Z, scale, store
                psZ = psSp.tile([1, S // 128], DT_F32, tag="psS")
                nc.tensor.matmul(psZ[:], ones_s[:, 0:1], w_stage[:], start=True, stop=True)
                z1 = smp.tile([1, 1], DT_F32)
                nc.vector.tensor_reduce(
                    z1[:], psZ[:], axis=mybir.AxisListType.X, op=mybir.AluOpType.add
                )
                psZb = psSp.tile([128, 1], DT_F32, tag="psS")
                nc.tensor.matmul(psZb[:], ones1_s[:], z1[:], start=True, stop=True)
                rz = smp.tile([128, 1], DT_F32)
                nc.vector.reciprocal(rz[:], psZb[:])
                wout = outp.tile([128, S // 128], DT_F32)
                nc.vector.tensor_scalar_mul(wout[:], w_stage[:], rz[:])
                ctxout = outp.tile([1, H], DT_F32)
                nc.vector.tensor_scalar_mul(ctxout[:], psC[:], rz[0:1, :])
                nc.sync.dma_start(
                    wts_d[b].rearrange("(c p) -> p c", p=128), wout[:]
                )
                nc.sync.dma_start(ctx_d[b : b + 1, :], ctxout[:])

    nc.compile()
    return nc


def _prep_inputs(lstm_output, mask, W_w, W_b, v_w):
    x = np.asarray(lstm_output, dtype=np.float32)
    xb = x.astype(bf16)                                   # (B, S, H)
    # xtp[b, p, hc, s] = x[b, s, hc*128+p]
    xtp = np.ascontiguousarray(
        xb.reshape(B, S, NC, 128).transpose(0, 3, 2, 1)
    )
    # xnp[b, p, c, h] = x[b, c*128+p, h]
    xnp = np.ascontiguousarray(
        xb.reshape(B, S // 128, 128, H).transpose(0, 2, 1, 3)
    )
    wt = np.ascontiguousarray(np.asarray(W_w, np.float32).T.reshape(NC, 128, H)).astype(bf16)
    biasc = np.ascontiguousarray(np.asarray(W_b, np.float32).reshape(NC, 128))
    vwc = np.ascontiguousarray(np.asarray(v_w, np.float32).reshape(NC, 128)).astype(bf16)
    madd = np.where(np.asarray(mask), np.float32(0.0), np.float32(-1e9)).astype(np.float32)
    # madd_d[b, p, c] = madd[b, c*128+p]
    madd = np.ascontiguousarray(madd.reshape(B, S // 128, 128).transpose(0, 2, 1))

    in_maps = []
    for c in range(NCORES):
        sl = slice(c * BPC, (c + 1) * BPC)
        in_maps.append(
            {
                "xtp": np.ascontiguousarray(xtp[sl]),
                "xnp": np.ascontiguousarray(xnp[sl]),
                "wt": wt,
                "bias": biasc,
                "vw": vwc,
                "madd": np.ascontiguousarray(madd[sl]),
            }
        )
    return in_maps


def kernel(lstm_output, mask, W_w, W_b, v_w):
    if "nc" not in _CACHE:
        _CACHE["nc"] = build()
    nc = _CACHE["nc"]
    in_maps = _prep_inputs(lstm_output, mask, W_w, W_b, v_w)
    res = run_bass_kernel_spmd(nc, in_maps, core_ids=list(range(NCORES)))
    ctx = np.concatenate([res.results[i]["ctx"] for i in range(NCORES)], axis=0)
    wts = np.concatenate([res.results[i]["wts"] for i in range(NCORES)], axis=0)
    return ctx.astype(np.float32), wts.astype(np.float32)
